# revision 1
# baseline (speedup 1.0000x reference)
"""Trainium2 Bass kernel for nn_ConvFilter (geometric-series conv filter).

Math (per batch b, output position l, feature f):
    t[o,l]  = sum_{i,k} conv_w[o,i,k] * x[l+k,i]          (valid conv, L=S-K+1)
    tau     = sigmoid(t + bias)
    out     = (sum_i tau^(7-i) * x[l+i,f]) / (sum_i tau^i)

Implementation:
  * transposed layout [feature, seq] on device; host pre/post-transposes.
  * conv: 16 accumulating fp32r matmuls per 512-wide l-tile (full-rate PE);
    two overlapping l-tiles (0 and L-512) since fp32r needs even free sizes.
  * numerator in fp16 on DVE (2x packed mode) with powers from ACT Squares:
        q_j = tau*x_{2j} + x_{2j+1}
        N   = (q0*T2 + q1)*T4 + (q2*T2 + q3),   T2 = tau^2, T4 = tau^4
    odd-shift windows read a one-element-shifted fp16 copy of x so every
    window stays 4-byte aligned (keeps the DVE 2x mode).
  * denominator fp32: D = (1+tau)(1+tau^2)(1+tau^4) as one custom DVE op,
    reciprocal via reciprocal_approx_fast; out = N * r (fp32).
  * engine split: ACT does sigmoid/converts/squares, DVE the main chain,
    GPSIMD the independent side-branch, PE only matmuls.
  * data-parallel over batch: 8 batches/core on 8 cores, weights replicated.
"""

import numpy as np
from contextlib import ExitStack

import concourse.bass as bass
import concourse.tile as tile
from concourse import bacc, mybir
from concourse.bass_utils import run_bass_kernel_spmd
from concourse import dve_ops
from concourse.dve_ops import DveOp
from concourse.dve_spec import Spec, Src0, Src1, lower, sq, One, _has_src1
from concourse.dve_uop import DveOpSpec

B, S, F, K = 64, 1024, 256, 8
L = S - K + 1  # 1017
NCORES = 8
BPC = B // NCORES
P = 128
NFB = F // P  # 2 feature blocks
LT = 512      # matmul l-tile width (one PSUM bank)
LE = L + 1    # even fp16 elementwise width (DVE 2x mode needs even counts)


def _register_op(name, spec, subdim=False):
    for existing in dve_ops.OPS:
        if existing.name == name:
            return existing
    shas = {}
    for ver in ("v3", "v4"):
        tmp = DveOpSpec(name=name, opcode=0, uops=lower(spec, ver=ver),
                        rd1_en=_has_src1(spec))
        shas[ver] = tmp.sha(ver)
    op = DveOp(name, spec, subdim=subdim, uops_sha=shas)
    dve_ops.OPS.append(op)
    dve_ops.CUSTOM_DVE_SPECS[name] = spec
    dve_ops._SUB_OPCODE_FOR_NAME[name] = (
        dve_ops._CUSTOM_DVE_ROW_BASE + len(dve_ops.OPS) - 1
    )
    assert dve_ops._SUB_OPCODE_FOR_NAME[name] < 0x20
    return op


def _get_ops():
    _t2 = sq(Src0)
    _t4 = sq(_t2)
    denom_spec = Spec(
        body=(Src0 + One) * (_t2 + One) * (_t4 + One),
        reference=lambda in0, in1, s0, s1, imm2: (
            (1.0 + in0) * (1.0 + in0 * in0) * (1.0 + in0 ** 4)
        ).astype(np.float32),
    )
    return _register_op("ANT_CF_DENOM", denom_spec)


def build_module():
    DENOM_OP = _get_ops()
    f32 = mybir.dt.float32
    f32r = mybir.dt.float32r
    f16 = mybir.dt.float16
    TT = mybir.AluOpType
    SIG = mybir.ActivationFunctionType.Sigmoid
    SQU = mybir.ActivationFunctionType.Square
    CPY = mybir.ActivationFunctionType.Copy

    nc = bacc.Bacc("TRN2", target_bir_lowering=False, debug=False,
                   enable_asserts=False, num_devices=NCORES)

    xt_d = nc.dram_tensor("xt", [BPC, NFB, P, S], f32, kind="ExternalInput").ap()
    wt_d = nc.dram_tensor("wt", [K, NFB, P, F], f32, kind="ExternalInput").ap()
    cb_d = nc.dram_tensor("cb", [F, 1], f32, kind="ExternalInput").ap()
    yt_d = nc.dram_tensor("yt", [BPC, NFB, P, L], f32, kind="ExternalOutput").ap()

    with tile.TileContext(nc) as tc, ExitStack() as ctx:
        wpool = ctx.enter_context(tc.tile_pool(name="w", bufs=1))
        xpool = ctx.enter_context(tc.tile_pool(name="x", bufs=2))
        tpool = ctx.enter_context(tc.tile_pool(name="t", bufs=3))
        qpool = ctx.enter_context(tc.tile_pool(name="q", bufs=2))
        opool = ctx.enter_context(tc.tile_pool(name="o", bufs=2))
        ppool = ctx.enter_context(tc.tile_pool(name="p", bufs=2, space="PSUM"))

        # weights + bias: loaded once, live forever
        w_sb = []
        for k in range(K):
            row = []
            for ic in range(NFB):
                t = wpool.tile([P, F], f32r, tag=f"w{k}{ic}")
                nc.sync.dma_start(t[:], wt_d[k, ic].bitcast(f32r))
                row.append(t)
            w_sb.append(row)
        bias_sb = wpool.tile([P, NFB], f32, tag="bias")
        nc.sync.dma_start(
            bias_sb[:], cb_d.rearrange("(ob p) one -> p (ob one)", p=P))

        for b in range(BPC):
            # x^T, both feature blocks side by side: [128, 2048] fp32r
            xt = xpool.tile([P, NFB * S], f32r, tag="xt")
            for ic in range(NFB):
                nc.sync.dma_start(xt[:, ic * S:(ic + 1) * S],
                                  xt_d[b, ic].bitcast(f32r))
            # fp16 copies for the elementwise chain (xh_odd = x shifted by 1
            # so odd-shift windows stay 4B-aligned for the DVE 2x mode)
            xf = xt[:].bitcast(f32)
            xh = xpool.tile([P, NFB * S], f16, tag="xh")
            nc.scalar.activation(xh[:], xf, CPY)
            xho = xpool.tile([P, NFB * S], f16, tag="xho")
            nc.scalar.activation(xho[:, :NFB * S - 1],
                                 xt[:, 1:NFB * S].bitcast(f32), CPY)

            # conv -> tau, per output-feature block; 4 PSUM tiles per batch,
            # matmuls ordered weight-major so each LDWEIGHTS serves 2 MMs.
            pss = {}
            for ob in range(NFB):
                for li, l0 in enumerate((0, L - LT)):
                    pss[(ob, li)] = ppool.tile([P, LT], f32, tag=f"ps{ob}{li}",
                                               name=f"ps{ob}{li}_{b}")
            for ic in range(NFB):
                for k in range(K):
                    first = (ic == 0 and k == 0)
                    last = (ic == NFB - 1 and k == K - 1)
                    for ob in range(NFB):
                        for li, l0 in enumerate((0, L - LT)):
                            nc.tensor.matmul(
                                pss[(ob, li)][:],
                                w_sb[k][ic][:, ob * P:(ob + 1) * P],
                                xt[:, ic * S + l0 + k: ic * S + l0 + k + LT],
                                start=first, stop=last,
                            )

            # tau (fp16, both obs in one [128, 2048] tile at cols ob*1024)
            W2 = NFB * S
            tau = tpool.tile([P, W2], f16, tag="tau")
            for ob in range(NFB):
                for li, l0 in enumerate((0, L - LT)):
                    nc.scalar.activation(
                        tau[:, ob * S + l0: ob * S + l0 + LT],
                        pss[(ob, li)][:], SIG,
                        bias=bias_sb[:, ob:ob + 1], scale=1.0)
            t2 = tpool.tile([P, W2], f16, tag="t2")
            nc.scalar.activation(t2[:], tau[:], SQU)
            t4 = tpool.tile([P, W2], f16, tag="t4")
            nc.scalar.activation(t4[:], t2[:], SQU)

            def pair(t, off=0):
                return t[:].rearrange("p (c n) -> p c n", c=2)[:, :, off:off + LE]

            th, t2p, t4p = pair(tau), pair(t2), pair(t4)

            def weven(i):
                return pair(xh, i)

            def wodd(i):  # i odd; the shifted copy at i-1 keeps alignment
                return pair(xho, i - 1)

            # numerator chain, all fp16 2x-mode on DVE (GPSIMD unused: its
            # concurrent SBUF traffic halves DVE throughput via port sharing)
            u0 = qpool.tile([P, W2], f16, tag="u")
            nc.vector.tensor_tensor(pair(u0), th, weven(0), TT.mult)
            q0 = qpool.tile([P, W2], f16, tag="q0")
            nc.vector.tensor_tensor(pair(q0), pair(u0), wodd(1), TT.add)
            m0 = qpool.tile([P, W2], f16, tag="m")
            nc.vector.tensor_tensor(pair(m0), pair(q0), t2p, TT.mult)

            u1 = qpool.tile([P, W2], f16, tag="u")
            nc.vector.tensor_tensor(pair(u1), th, weven(2), TT.mult)
            q1 = qpool.tile([P, W2], f16, tag="q1")
            nc.vector.tensor_tensor(pair(q1), pair(u1), wodd(3), TT.add)
            h0 = qpool.tile([P, W2], f16, tag="hh")
            nc.vector.tensor_tensor(pair(h0), pair(m0), pair(q1), TT.add)
            m1 = qpool.tile([P, W2], f16, tag="m")
            nc.vector.tensor_tensor(pair(m1), pair(h0), t4p, TT.mult)

            u2 = qpool.tile([P, W2], f16, tag="u")
            nc.vector.tensor_tensor(pair(u2), th, weven(4), TT.mult)
            q2 = qpool.tile([P, W2], f16, tag="q2")
            nc.vector.tensor_tensor(pair(q2), pair(u2), wodd(5), TT.add)
            h1 = qpool.tile([P, W2], f16, tag="hh")
            nc.vector.tensor_tensor(pair(h1), pair(q2), t2p, TT.mult)

            u3 = qpool.tile([P, W2], f16, tag="u")
            nc.vector.tensor_tensor(pair(u3), th, weven(6), TT.mult)
            q3 = qpool.tile([P, W2], f16, tag="q3")
            nc.vector.tensor_tensor(pair(q3), pair(u3), wodd(7), TT.add)
            h2 = qpool.tile([P, W2], f16, tag="h2")
            nc.vector.tensor_tensor(pair(h2), pair(h1), pair(q3), TT.add)

            nh = qpool.tile([P, W2], f16, tag="nh")
            nc.vector.tensor_tensor(pair(nh), pair(m1), pair(h2), TT.add)

            # denominator + division (fp32 tail), full-width incl. junk cols
            d = opool.tile([P, W2], f32, tag="d")
            nc.vector._custom_dve(DENOM_OP, out=d[:], in0=tau[:])
            # r in fp16 (direct _custom_dve: wrapper insists on fp32 out,
            # but the NR math runs in-pipe at fp32; only the store rounds)
            from concourse.dve_ops import (RECIPROCAL_APPROX_FAST,
                                           RECIP_APPROX_FAST_CONSTS as RC)
            r = opool.tile([P, W2], f16, tag="r")
            nc.vector._custom_dve(RECIPROCAL_APPROX_FAST, out=r[:], in0=d[:],
                                  s0=RC["s0"], s1=RC["s1"], imm2=RC["imm2"])
            oh = opool.tile([P, W2], f16, tag="oh")
            nc.vector.tensor_tensor(pair(oh), pair(nh), pair(r), TT.mult)
            of = opool.tile([P, W2], f32, tag="of")
            nc.scalar.activation(of[:], oh[:], CPY)
            for ob in range(NFB):
                nc.sync.dma_start(yt_d[b, ob], of[:, ob * S: ob * S + L])

    nc.compile()
    return nc


_NC = None


def _get_nc():
    global _NC
    if _NC is None:
        _NC = build_module()
    return _NC


def prep_inputs(x, conv_w, conv_b):
    xt = np.ascontiguousarray(
        x.transpose(0, 2, 1)).astype(np.float32, copy=False)
    xt = xt.reshape(B, NFB, P, S)
    wt = np.ascontiguousarray(
        conv_w.transpose(2, 1, 0)).astype(np.float32, copy=False)
    wt = wt.reshape(K, NFB, P, F)
    cb = np.ascontiguousarray(conv_b, dtype=np.float32).reshape(F, 1)
    return xt, wt, cb


def make_in_maps(x, conv_w, conv_b):
    xt, wt, cb = prep_inputs(x, conv_w, conv_b)
    return [
        {"xt": xt[c * BPC:(c + 1) * BPC], "wt": wt, "cb": cb}
        for c in range(NCORES)
    ]


def gather_output(results):
    out = np.empty((B, L, F), np.float32)
    for c in range(NCORES):
        yt = results[c]["yt"]  # [BPC, NFB, P, L]
        out[c * BPC:(c + 1) * BPC] = (
            yt.transpose(0, 3, 1, 2).reshape(BPC, L, F))
    return out


def kernel(x, conv_w, conv_b):
    nc = _get_nc()
    in_maps = make_in_maps(x, conv_w, conv_b)
    res = run_bass_kernel_spmd(nc, in_maps, core_ids=list(range(NCORES)))
    return gather_output(res.results)

